# revision 8
# baseline (speedup 1.0000x reference)
"""Multi-head attention (B=2, S=2048, D=1024, H=16, DK=64) with RoPE and
causal masking, sharded over 8 NeuronCores as (batch x head-group):
core c handles batch c//4 and heads 4*(c%4) .. 4*(c%4)+4.

Per-core dataflow (all layouts chosen so no on-device transposes are needed):
  - host pre-transposes activations X^T [D, S] and weight slices.
  - projections produce Q^T/K^T in an "A layout" per 128-partition tile
    ([h0e|h1e|h0o|h1o]: RoPE even/odd dims grouped) via PE matmul,
    evicted from PSUM with fused bias add (DVE tensor_scalar_add).
  - RoPE runs full-width DVE ops on the A tiles and re-packs into the
    "B layout" ([h0e'|h0o'|h1e'|h1o']: head-contiguous, 64 dims/head).
  - scores_t[k,q] = K_B.T @ Q_B per (qtile 512, kblock 128, head), K_c=64.
    Softmax without max-subtraction (scores ~ N(0,1), safe in fp32):
    P = exp(0.125*s + addpat) where addpat is 0/-1e30; causal blocks above
    the diagonal are skipped entirely.
  - AV: lhsT = [V | ones] (M=65) stationary, rhs = P_t moving; PSUM
    accumulates over kblocks; row 64 gives the softmax denominator l.
  - normalize: O^T = AV[0:64] * broadcast(1/l) (broadcast via K_c=1 matmul).
  - output projection: partial^T[j,s] = woT.T @ O^T; host sums the 4
    partials per batch, transposes, and adds bo.

The whole thing is emitted as one software pipeline over the 4 q s-tiles:
projections(st) -> RoPE(st) -> attention(qt=st) -> out-proj(st), so Tile
overlaps DMA/PE/DVE/ACT across phases.
"""
import numpy as np

B, S, D, H, DK = 2, 2048, 1024, 16, 64
NCORES = 8
HPC = 4            # heads per core
DH = HPC * DK      # 256 local head dims
QT = 512           # q tile (free dim of scores matmul)
KB = 128           # k block (partition dim of scores)
NQT = S // QT      # 4
NKB = S // KB      # 16
NDB = D // 128     # 8 d-blocks for projections
NEG = -1.0e30

_cache = {}


def _build_nc(causal: bool):
    from contextlib import ExitStack
    import concourse.bass as bass
    import concourse.tile as tile
    from concourse import bacc, mybir
    from concourse._compat import with_exitstack

    F32 = mybir.dt.float32
    AF = mybir.ActivationFunctionType
    OP = mybir.AluOpType

    nc = bacc.Bacc(None, target_bir_lowering=False, debug=False)

    xqT_d = nc.dram_tensor("xqT", [D, S], F32, kind="ExternalInput")
    xkT_d = nc.dram_tensor("xkT", [D, S], F32, kind="ExternalInput")
    xvT_d = nc.dram_tensor("xvT", [D, S], F32, kind="ExternalInput")
    wqT_d = nc.dram_tensor("wqT", [D, DH], F32, kind="ExternalInput")
    wkT_d = nc.dram_tensor("wkT", [D, DH], F32, kind="ExternalInput")
    wvT_d = nc.dram_tensor("wvT", [D, DH], F32, kind="ExternalInput")
    bqA_d = nc.dram_tensor("bqA", [DH], F32, kind="ExternalInput")
    bkA_d = nc.dram_tensor("bkA", [DH], F32, kind="ExternalInput")
    bv_d = nc.dram_tensor("bv", [DH], F32, kind="ExternalInput")
    woT_d = nc.dram_tensor("woT", [DH, D], F32, kind="ExternalInput")
    sin_d = nc.dram_tensor("sin4", [128, S], F32, kind="ExternalInput")
    cos_d = nc.dram_tensor("cos4", [128, S], F32, kind="ExternalInput")
    if causal:
        pat_d = nc.dram_tensor("addpat", [KB, 4, QT], F32, kind="ExternalInput")
    else:
        pat_d = nc.dram_tensor("amaskT", [S, S], F32, kind="ExternalInput")
    out_d = nc.dram_tensor("outT", [D, S], F32, kind="ExternalOutput")

    @with_exitstack
    def emit(ctx: ExitStack, tc: tile.TileContext):
        nc = tc.nc
        singles = ctx.enter_context(tc.tile_pool(name="singles", bufs=1))
        xpool = ctx.enter_context(tc.tile_pool(name="x", bufs=3))
        wpool = ctx.enter_context(tc.tile_pool(name="w", bufs=8))
        ropet = ctx.enter_context(tc.tile_pool(name="ropet", bufs=2))
        bpool = ctx.enter_context(tc.tile_pool(name="bpool", bufs=1))
        ppool = ctx.enter_context(tc.tile_pool(name="ppool", bufs=6))
        opool = ctx.enter_context(tc.tile_pool(name="opool", bufs=3))
        if not causal:
            ampool = ctx.enter_context(tc.tile_pool(name="ampool", bufs=3))

        ps_proj = ctx.enter_context(tc.tile_pool(name="ps_proj", bufs=2, space="PSUM"))
        ps_sc = ctx.enter_context(tc.tile_pool(name="ps_sc", bufs=2, space="PSUM"))
        ps_av = ctx.enter_context(tc.tile_pool(name="ps_av", bufs=4, space="PSUM"))

        # ---- constants ----
        sin4 = singles.tile([128, S], F32)
        cos4 = singles.tile([128, S], F32)
        nc.sync.dma_start(out=sin4, in_=sin_d[:])
        nc.sync.dma_start(out=cos4, in_=cos_d[:])
        ones65 = singles.tile([65, 64], F32)
        nc.vector.memset(ones65, 1.0)
        if causal:
            addpat = singles.tile([KB, 4, QT], F32)
            nc.sync.dma_start(out=addpat, in_=pat_d[:])

        bq_sb = singles.tile([128, 2], F32)
        bk_sb = singles.tile([128, 2], F32)
        nc.sync.dma_start(out=bq_sb, in_=bqA_d.rearrange("(t p) -> p t", p=128))
        nc.sync.dma_start(out=bk_sb, in_=bkA_d.rearrange("(t p) -> p t", p=128))
        bv_bc = singles.tile([128, DH], F32)
        nc.sync.dma_start(
            out=bv_bc,
            in_=bass.AP(tensor=bv_d[:].tensor, offset=0, ap=[[0, 128], [1, DH]]),
        )

        woT_sb = [singles.tile([128, D], F32, tag=f"woT{mb}", name=f"woT{mb}") for mb in range(2)]
        for mb in range(2):
            nc.sync.dma_start(out=woT_sb[mb], in_=woT_d[mb * 128:(mb + 1) * 128, :])

        # weights resident (8 d-block tiles per tensor; slots recycled q->k->v)
        wq = [wpool.tile([128, DH], F32, tag="wt", name=f"wq{i}") for i in range(NDB)]
        for db in range(NDB):
            nc.sync.dma_start(out=wq[db], in_=wqT_d[db * 128:(db + 1) * 128, :])
        wk = [wpool.tile([128, DH], F32, tag="wt2", name=f"wk{i}") for i in range(NDB)]
        for db in range(NDB):
            nc.sync.dma_start(out=wk[db], in_=wkT_d[db * 128:(db + 1) * 128, :])
        wv = [wpool.tile([128, DH], F32, tag="wt3", name=f"wv{i}") for i in range(NDB)]
        for db in range(NDB):
            nc.sync.dma_start(out=wv[db], in_=wvT_d[db * 128:(db + 1) * 128, :])

        # persistent activations (post-RoPE Q/K and attention output, T layout)
        qb = [bpool.tile([128, S], F32, tag=f"qb{t}", name=f"qb{t}") for t in range(2)]
        kb_t = [bpool.tile([128, S], F32, tag=f"kbt{t}", name=f"kbt{t}") for t in range(2)]
        oT = [bpool.tile([128, S], F32, tag=f"oT{t}", name=f"oT{t}") for t in range(2)]
        v_sb = singles.tile([128, NKB, HPC, DK + 1], F32)
        nc.vector.memset(v_sb[:, :, :, DK:DK + 1], 1.0)

        def proj_rope_qk(xd, ws, bias_sb, dst, st):
            # project into E/O-split PSUM groups (cols of ws are permuted as
            # [all-heads evens | all-heads odds]), apply bias + RoPE fully at
            # partitions 0..128, then DMA-scatter into head-contiguous dst.
            ps = [ps_proj.tile([128, QT], F32, tag="pp", name=f"pq{i}") for i in range(2)]
            for db in range(NDB):
                xt = xpool.tile([128, QT], F32, tag="xt")
                nc.sync.dma_start(
                    out=xt, in_=xd[db * 128:(db + 1) * 128, st * QT:(st + 1) * QT])
                for mt in range(2):
                    nc.tensor.matmul(
                        ps[mt][:, :],
                        lhsT=ws[db][:, mt * 128:(mt + 1) * 128],
                        rhs=xt[:, :],
                        start=(db == 0), stop=(db == NDB - 1),
                    )
            sl = slice(st * QT, (st + 1) * QT)
            xe = ropet.tile([128, QT], F32, tag="xe")
            xo = ropet.tile([128, QT], F32, tag="xo")
            nc.vector.tensor_scalar_add(xe, ps[0], bias_sb[:, 0:1])
            nc.vector.tensor_scalar_add(xo, ps[1], bias_sb[:, 1:2])
            t1 = ropet.tile([128, QT], F32, tag="t1")
            t2 = ropet.tile([128, QT], F32, tag="t2")
            t3 = ropet.tile([128, QT], F32, tag="t3")
            t4 = ropet.tile([128, QT], F32, tag="t4")
            nc.vector.tensor_mul(t1, xe, sin4[:, sl])
            nc.vector.tensor_mul(t2, xo, cos4[:, sl])
            nc.vector.tensor_mul(t3, xe, cos4[:, sl])
            nc.vector.tensor_mul(t4, xo, sin4[:, sl])
            nc.vector.tensor_sub(t3, t3, t4)   # evens'
            nc.vector.tensor_add(t1, t1, t2)   # odds'
            for g in range(4):                 # scatter per head
                t, hh = g // 2, g % 2
                nc.sync.dma_start(
                    out=dst[t][64 * hh:64 * hh + 32, sl],
                    in_=t3[32 * g:32 * g + 32, :])
                nc.sync.dma_start(
                    out=dst[t][64 * hh + 32:64 * hh + 64, sl],
                    in_=t1[32 * g:32 * g + 32, :])

        def proj_v(st):
            # V rows [s in st*QT..(st+1)*QT) -> kblocks 4st..4st+4
            for half in range(2):
                s0 = st * QT + half * 256
                ps = [ps_proj.tile([128, DH], F32, tag="pp", name=f"pv{i}") for i in range(2)]
                for db in range(NDB):
                    xt = xpool.tile([128, 256], F32, tag="xv")
                    nc.sync.dma_start(
                        out=xt, in_=xvT_d[db * 128:(db + 1) * 128, s0:s0 + 256])
                    for kbl in range(2):
                        nc.tensor.matmul(
                            ps[kbl][:, :],
                            lhsT=xt[:, kbl * 128:(kbl + 1) * 128],
                            rhs=wv[db][:, :],
                            start=(db == 0), stop=(db == NDB - 1),
                        )
                for kbl in range(2):
                    kb = s0 // 128 + kbl
                    nc.vector.tensor_add(
                        v_sb[:, kb, :, 0:DK],
                        ps[kbl].rearrange("p (h d) -> p h d", h=HPC),
                        bv_bc.rearrange("p (h d) -> p h d", h=HPC),
                    )

        for st in range(NQT):
            # --- projections for this s-tile ---
            proj_rope_qk(xqT_d, wq, bq_sb, qb, st)
            proj_rope_qk(xkT_d, wk, bk_sb, kb_t, st)
            proj_v(st)

            # --- attention for qt = st ---
            qt = st
            nkb = 4 * qt + 4 if causal else NKB
            av_ps = [ps_av.tile([DK + 1, QT], F32, tag="av", name=f"av{i}") for i in range(HPC)]
            for kb in range(nkb):
                if not causal:
                    am = ampool.tile([KB, QT], F32, tag="am")
                    nc.sync.dma_start(
                        out=am,
                        in_=pat_d[kb * KB:(kb + 1) * KB, qt * QT:(qt + 1) * QT])
                diag = causal and (kb >= 4 * qt)
                for h in range(HPC):
                    t, hh = h // 2, h % 2
                    sc = ps_sc.tile([KB, QT], F32, tag="sc")
                    nc.tensor.matmul(
                        sc[:, :],
                        lhsT=kb_t[t][64 * hh:64 * hh + 64, kb * KB:(kb + 1) * KB],
                        rhs=qb[t][64 * hh:64 * hh + 64, qt * QT:(qt + 1) * QT],
                        start=True, stop=True,
                    )
                    pt = ppool.tile([KB, QT], F32, tag="pt")
                    if causal and not diag:
                        nc.scalar.activation(out=pt, in_=sc, func=AF.Exp, scale=0.125)
                    else:
                        pat = (addpat[:, kb - 4 * qt, :] if causal else am)
                        nc.vector.scalar_tensor_tensor(
                            out=pt, in0=sc, scalar=0.125, in1=pat,
                            op0=OP.mult, op1=OP.add)
                        nc.scalar.activation(out=pt, in_=pt, func=AF.Exp)
                    nc.tensor.matmul(
                        av_ps[h][:, :],
                        lhsT=v_sb[:, kb, h, :],
                        rhs=pt[:, :],
                        start=(kb == 0), stop=(kb == nkb - 1),
                    )
            # normalize each head -> O^T
            for h in range(HPC):
                t, hh = h // 2, h % 2
                recip = opool.tile([65, QT], F32, tag="recip")
                nc.vector.reciprocal(recip[64:65, :], av_ps[h][DK:DK + 1, :])
                bc_ps = ps_sc.tile([64, QT], F32, tag="sc")
                nc.tensor.matmul(bc_ps[:, :], lhsT=ones65[64:65, :],
                                 rhs=recip[64:65, :], start=True, stop=True)
                bc_sb = opool.tile([64, QT], F32, tag="bc")
                nc.scalar.activation(out=bc_sb, in_=bc_ps, func=AF.Copy)
                otmp = opool.tile([64, QT], F32, tag="otmp")
                nc.vector.tensor_mul(otmp, av_ps[h][0:DK, :], bc_sb)
                nc.sync.dma_start(
                    out=oT[t][64 * hh:64 * hh + 64, qt * QT:(qt + 1) * QT],
                    in_=otmp)
            # --- output projection for this s-tile ---
            for jb in range(8):
                po = ps_sc.tile([128, QT], F32, tag="sc")
                for mb in range(2):
                    nc.tensor.matmul(
                        po[:, :],
                        lhsT=woT_sb[mb][:, jb * 128:(jb + 1) * 128],
                        rhs=oT[mb][:, qt * QT:(qt + 1) * QT],
                        start=(mb == 0), stop=(mb == 1),
                    )
                ob = opool.tile([128, QT], F32, tag="ob")
                nc.scalar.activation(out=ob, in_=po, func=AF.Copy)
                nc.sync.dma_start(
                    out=out_d[jb * 128:(jb + 1) * 128, qt * QT:(qt + 1) * QT],
                    in_=ob)

    with tile.TileContext(nc) as tc:
        emit(tc)
    nc.compile()
    return nc


def _host_prep(inputs):
    k, q, v = inputs["k"], inputs["q"], inputs["v"]
    mask, sin, cos = inputs["mask"], inputs["sin"], inputs["cos"]
    Wq, bq = inputs["Wq"], inputs["bq"]
    Wk, bk = inputs["Wk"], inputs["bk"]
    Wv, bv = inputs["Wv"], inputs["bv"]
    Wo = inputs["Wo"]

    causal = bool(np.array_equal(
        np.asarray(mask[0, 0]),
        np.tril(np.ones((S, S), np.asarray(mask).dtype))))

    sinT = np.ascontiguousarray(np.asarray(sin[0, 0]).T.astype(np.float32))
    cosT = np.ascontiguousarray(np.asarray(cos[0, 0]).T.astype(np.float32))
    sin4 = np.ascontiguousarray(np.tile(sinT, (4, 1)))
    cos4 = np.ascontiguousarray(np.tile(cosT, (4, 1)))

    if causal:
        ki = np.arange(KB)[:, None]
        qi = np.arange(QT)[None, :]
        pat = np.stack([
            np.where(128 * j + ki <= qi, 0.0, NEG).astype(np.float32)
            for j in range(4)])
        pat = np.ascontiguousarray(pat.transpose(1, 0, 2))
        amaskT = None
    else:
        pat = None
        amaskT = np.ascontiguousarray(
            np.where(np.asarray(mask[0, 0]).T == 0, NEG, 0.0).astype(np.float32))

    # E/O permutation of the 256 local head dims:
    # [all heads' even dims | all heads' odd dims]
    ev, od = np.arange(0, DK, 2), np.arange(1, DK, 2)
    perm = np.concatenate(
        [64 * h + ev for h in range(4)] + [64 * h + od for h in range(4)])

    xT = {}
    for name, x in (("q", q), ("k", k), ("v", v)):
        for b in range(B):
            xT[(name, b)] = np.ascontiguousarray(
                np.asarray(x[b]).T.astype(np.float32))

    in_maps = []
    for c in range(NCORES):
        b, hg = c // 4, c % 4
        rows = slice(hg * DH, (hg + 1) * DH)
        wqT = np.ascontiguousarray(np.asarray(Wq)[rows, :].T[:, perm])
        wkT = np.ascontiguousarray(np.asarray(Wk)[rows, :].T[:, perm])
        wvT = np.ascontiguousarray(np.asarray(Wv)[rows, :].T)
        woT = np.ascontiguousarray(np.asarray(Wo)[:, rows].T)
        m = dict(
            xqT=xT[("q", b)], xkT=xT[("k", b)], xvT=xT[("v", b)],
            wqT=wqT.astype(np.float32), wkT=wkT.astype(np.float32),
            wvT=wvT.astype(np.float32), woT=woT.astype(np.float32),
            bqA=np.ascontiguousarray(np.asarray(bq)[rows][perm]).astype(np.float32),
            bkA=np.ascontiguousarray(np.asarray(bk)[rows][perm]).astype(np.float32),
            bv=np.ascontiguousarray(np.asarray(bv)[rows]).astype(np.float32),
            sin4=sin4, cos4=cos4,
        )
        if causal:
            m["addpat"] = pat
        else:
            m["amaskT"] = amaskT
        in_maps.append(m)
    return causal, in_maps


def kernel(**inputs):
    from concourse.bass_utils import run_bass_kernel_spmd

    causal, in_maps = _host_prep(inputs)
    if causal not in _cache:
        _cache[causal] = _build_nc(causal)
    nc = _cache[causal]

    res = run_bass_kernel_spmd(nc, in_maps, core_ids=list(range(NCORES))).results

    bo = np.asarray(inputs["bo"]).astype(np.float32)
    out = np.empty((B, S, D), np.float32)
    for b in range(B):
        acc = res[4 * b]["outT"].astype(np.float32).copy()
        for c in range(4 * b + 1, 4 * b + 4):
            acc += res[c]["outT"]
        out[b] = acc.T + bo
    return out


# revision 9
# speedup vs baseline: 1.7010x; 1.7010x over previous
"""Multi-head attention (B=2, S=2048, D=1024, H=16, DK=64) with RoPE and
causal masking, sharded over 8 NeuronCores as (batch x head-group):
core c handles batch c//4 and heads 4*(c%4) .. 4*(c%4)+4.

Per-core dataflow (all layouts chosen so no on-device transposes are needed):
  - host pre-transposes activations X^T [D, S] and weight slices.
  - projections produce Q^T/K^T in an "A layout" per 128-partition tile
    ([h0e|h1e|h0o|h1o]: RoPE even/odd dims grouped) via PE matmul,
    evicted from PSUM with fused bias add (DVE tensor_scalar_add).
  - RoPE runs full-width DVE ops on the A tiles and re-packs into the
    "B layout" ([h0e'|h0o'|h1e'|h1o']: head-contiguous, 64 dims/head).
  - scores_t[k,q] = K_B.T @ Q_B per (qtile 512, kblock 128, head), K_c=64.
    Softmax without max-subtraction (scores ~ N(0,1), safe in fp32):
    P = exp(0.125*s + addpat) where addpat is 0/-1e30; causal blocks above
    the diagonal are skipped entirely.
  - AV: lhsT = [V | ones] (M=65) stationary, rhs = P_t moving; PSUM
    accumulates over kblocks; row 64 gives the softmax denominator l.
  - normalize: O^T = AV[0:64] * broadcast(1/l) (broadcast via K_c=1 matmul).
  - output projection: partial^T[j,s] = woT.T @ O^T; host sums the 4
    partials per batch, transposes, and adds bo.

The whole thing is emitted as one software pipeline over the 4 q s-tiles:
projections(st) -> RoPE(st) -> attention(qt=st) -> out-proj(st), so Tile
overlaps DMA/PE/DVE/ACT across phases.
"""
import numpy as np
import ml_dtypes

BF = ml_dtypes.bfloat16
B, S, D, H, DK = 2, 2048, 1024, 16, 64
NCORES = 8
HPC = 4            # heads per core
DH = HPC * DK      # 256 local head dims
QT = 512           # q tile (free dim of scores matmul)
KB = 128           # k block (partition dim of scores)
NQT = S // QT      # 4
NKB = S // KB      # 16
NDB = D // 128     # 8 d-blocks for projections
NEG = -1.0e30

_cache = {}


def _build_nc(causal: bool):
    from contextlib import ExitStack
    import concourse.bass as bass
    import concourse.tile as tile
    from concourse import bacc, mybir
    from concourse._compat import with_exitstack

    F32 = mybir.dt.float32
    BF16 = mybir.dt.bfloat16
    AF = mybir.ActivationFunctionType
    OP = mybir.AluOpType

    nc = bacc.Bacc(None, target_bir_lowering=False, debug=False)

    xqT_d = nc.dram_tensor("xqT", [D, S], BF16, kind="ExternalInput")
    xkT_d = nc.dram_tensor("xkT", [D, S], BF16, kind="ExternalInput")
    xvT_d = nc.dram_tensor("xvT", [D, S], BF16, kind="ExternalInput")
    wqT_d = nc.dram_tensor("wqT", [D, DH], BF16, kind="ExternalInput")
    wkT_d = nc.dram_tensor("wkT", [D, DH], BF16, kind="ExternalInput")
    wvT_d = nc.dram_tensor("wvT", [D, DH], BF16, kind="ExternalInput")
    bqA_d = nc.dram_tensor("bqA", [DH], F32, kind="ExternalInput")
    bkA_d = nc.dram_tensor("bkA", [DH], F32, kind="ExternalInput")
    bv_d = nc.dram_tensor("bv", [DH], F32, kind="ExternalInput")
    woT_d = nc.dram_tensor("woT", [DH, D], BF16, kind="ExternalInput")
    sin_d = nc.dram_tensor("sin4", [128, S], F32, kind="ExternalInput")
    cos_d = nc.dram_tensor("cos4", [128, S], F32, kind="ExternalInput")
    if causal:
        pat_d = nc.dram_tensor("addpat", [KB, 4, QT], F32, kind="ExternalInput")
    else:
        pat_d = nc.dram_tensor("amaskT", [S, S], F32, kind="ExternalInput")
    out_d = nc.dram_tensor("outT", [D, S], F32, kind="ExternalOutput")

    @with_exitstack
    def emit(ctx: ExitStack, tc: tile.TileContext):
        nc = tc.nc
        singles = ctx.enter_context(tc.tile_pool(name="singles", bufs=1))
        xpool = ctx.enter_context(tc.tile_pool(name="x", bufs=3))
        wpool = ctx.enter_context(tc.tile_pool(name="w", bufs=8))
        ropet = ctx.enter_context(tc.tile_pool(name="ropet", bufs=2))
        bpool = ctx.enter_context(tc.tile_pool(name="bpool", bufs=1))
        ppool = ctx.enter_context(tc.tile_pool(name="ppool", bufs=6))
        opool = ctx.enter_context(tc.tile_pool(name="opool", bufs=3))
        if not causal:
            ampool = ctx.enter_context(tc.tile_pool(name="ampool", bufs=3))

        ps_proj = ctx.enter_context(tc.tile_pool(name="ps_proj", bufs=2, space="PSUM"))
        ps_sc = ctx.enter_context(tc.tile_pool(name="ps_sc", bufs=2, space="PSUM"))
        ps_av = ctx.enter_context(tc.tile_pool(name="ps_av", bufs=4, space="PSUM"))

        # ---- constants ----
        sin4 = singles.tile([128, S], F32)
        cos4 = singles.tile([128, S], F32)
        nc.sync.dma_start(out=sin4, in_=sin_d[:])
        nc.sync.dma_start(out=cos4, in_=cos_d[:])
        ones65 = singles.tile([65, 64], F32)
        nc.vector.memset(ones65, 1.0)
        if causal:
            addpat = singles.tile([KB, 4, QT], F32)
            nc.sync.dma_start(out=addpat, in_=pat_d[:])

        bq_sb = singles.tile([128, 2], F32)
        bk_sb = singles.tile([128, 2], F32)
        nc.sync.dma_start(out=bq_sb, in_=bqA_d.rearrange("(t p) -> p t", p=128))
        nc.sync.dma_start(out=bk_sb, in_=bkA_d.rearrange("(t p) -> p t", p=128))
        bv_bc = singles.tile([128, DH], F32)
        nc.sync.dma_start(
            out=bv_bc,
            in_=bass.AP(tensor=bv_d[:].tensor, offset=0, ap=[[0, 128], [1, DH]]),
        )

        woT_sb = [singles.tile([128, D], BF16, tag=f"woT{mb}", name=f"woT{mb}") for mb in range(2)]
        for mb in range(2):
            nc.sync.dma_start(out=woT_sb[mb], in_=woT_d[mb * 128:(mb + 1) * 128, :])

        # weights resident (8 d-block tiles per tensor; slots recycled q->k->v)
        wq = [wpool.tile([128, DH], BF16, tag="wt", name=f"wq{i}") for i in range(NDB)]
        for db in range(NDB):
            nc.sync.dma_start(out=wq[db], in_=wqT_d[db * 128:(db + 1) * 128, :])
        wk = [wpool.tile([128, DH], BF16, tag="wt2", name=f"wk{i}") for i in range(NDB)]
        for db in range(NDB):
            nc.sync.dma_start(out=wk[db], in_=wkT_d[db * 128:(db + 1) * 128, :])
        wv = [wpool.tile([128, DH], BF16, tag="wt3", name=f"wv{i}") for i in range(NDB)]
        for db in range(NDB):
            nc.sync.dma_start(out=wv[db], in_=wvT_d[db * 128:(db + 1) * 128, :])

        # persistent activations (post-RoPE Q/K and attention output, T layout)
        qb = [bpool.tile([128, S], BF16, tag=f"qb{t}", name=f"qb{t}") for t in range(2)]
        kb_t = [bpool.tile([128, S], BF16, tag=f"kbt{t}", name=f"kbt{t}") for t in range(2)]
        oT = [bpool.tile([128, S], BF16, tag=f"oT{t}", name=f"oT{t}") for t in range(2)]
        v_sb = singles.tile([128, NKB, HPC, DK + 1], BF16)
        nc.vector.memset(v_sb[:, :, :, DK:DK + 1], 1.0)

        def proj_rope_qk(xd, ws, bias_sb, dst, st):
            # project into E/O-split PSUM groups (cols of ws are permuted as
            # [all-heads evens | all-heads odds]), apply bias + RoPE fully at
            # partitions 0..128, then DMA-scatter into head-contiguous dst.
            ps = [ps_proj.tile([128, QT], F32, tag="pp", name=f"pq{i}") for i in range(2)]
            for db in range(NDB):
                xt = xpool.tile([128, QT], BF16, tag="xt")
                nc.sync.dma_start(
                    out=xt, in_=xd[db * 128:(db + 1) * 128, st * QT:(st + 1) * QT])
                for mt in range(2):
                    nc.tensor.matmul(
                        ps[mt][:, :],
                        lhsT=ws[db][:, mt * 128:(mt + 1) * 128],
                        rhs=xt[:, :],
                        start=(db == 0), stop=(db == NDB - 1),
                    )
            sl = slice(st * QT, (st + 1) * QT)
            t1 = ropet.tile([128, QT], F32, tag="t1")
            t2 = ropet.tile([128, QT], F32, tag="t2")
            t3 = ropet.tile([128, QT], F32, tag="t3")
            t4 = ropet.tile([128, QT], F32, tag="t4")
            # t = (psum + bias) * sin/cos, bias fused via scalar_tensor_tensor
            nc.vector.scalar_tensor_tensor(
                out=t1, in0=ps[0], scalar=bias_sb[:, 0:1], in1=sin4[:, sl],
                op0=OP.add, op1=OP.mult)
            nc.vector.scalar_tensor_tensor(
                out=t3, in0=ps[0], scalar=bias_sb[:, 0:1], in1=cos4[:, sl],
                op0=OP.add, op1=OP.mult)
            nc.vector.scalar_tensor_tensor(
                out=t2, in0=ps[1], scalar=bias_sb[:, 1:2], in1=cos4[:, sl],
                op0=OP.add, op1=OP.mult)
            nc.vector.scalar_tensor_tensor(
                out=t4, in0=ps[1], scalar=bias_sb[:, 1:2], in1=sin4[:, sl],
                op0=OP.add, op1=OP.mult)
            ev = ropet.tile([128, QT], BF16, tag="ev")
            odd = ropet.tile([128, QT], BF16, tag="odd")
            nc.vector.tensor_sub(ev, t3, t4)    # evens'
            nc.vector.tensor_add(odd, t1, t2)   # odds'
            for g in range(4):                  # scatter per head
                t, hh = g // 2, g % 2
                nc.sync.dma_start(
                    out=dst[t][64 * hh:64 * hh + 32, sl],
                    in_=ev[32 * g:32 * g + 32, :])
                nc.sync.dma_start(
                    out=dst[t][64 * hh + 32:64 * hh + 64, sl],
                    in_=odd[32 * g:32 * g + 32, :])

        def proj_v(st):
            # V rows [s in st*QT..(st+1)*QT) -> kblocks 4st..4st+4
            for half in range(2):
                s0 = st * QT + half * 256
                ps = [ps_proj.tile([128, DH], F32, tag="pp", name=f"pv{i}") for i in range(2)]
                for db in range(NDB):
                    xt = xpool.tile([128, 256], BF16, tag="xv")
                    nc.sync.dma_start(
                        out=xt, in_=xvT_d[db * 128:(db + 1) * 128, s0:s0 + 256])
                    for kbl in range(2):
                        nc.tensor.matmul(
                            ps[kbl][:, :],
                            lhsT=xt[:, kbl * 128:(kbl + 1) * 128],
                            rhs=wv[db][:, :],
                            start=(db == 0), stop=(db == NDB - 1),
                        )
                for kbl in range(2):
                    kb = s0 // 128 + kbl
                    nc.vector.tensor_add(
                        v_sb[:, kb, :, 0:DK],
                        ps[kbl].rearrange("p (h d) -> p h d", h=HPC),
                        bv_bc.rearrange("p (h d) -> p h d", h=HPC),
                    )

        for st in range(NQT):
            # --- projections for this s-tile ---
            proj_rope_qk(xqT_d, wq, bq_sb, qb, st)
            proj_rope_qk(xkT_d, wk, bk_sb, kb_t, st)
            proj_v(st)

            # --- attention for qt = st ---
            qt = st
            nkb = 4 * qt + 4 if causal else NKB
            av_ps = [ps_av.tile([DK + 1, QT], F32, tag="av", name=f"av{i}") for i in range(HPC)]
            for kb in range(nkb):
                if not causal:
                    am = ampool.tile([KB, QT], F32, tag="am")
                    nc.sync.dma_start(
                        out=am,
                        in_=pat_d[kb * KB:(kb + 1) * KB, qt * QT:(qt + 1) * QT])
                diag = causal and (kb >= 4 * qt)
                for h in range(HPC):
                    t, hh = h // 2, h % 2
                    sc = ps_sc.tile([KB, QT], F32, tag="sc")
                    nc.tensor.matmul(
                        sc[:, :],
                        lhsT=kb_t[t][64 * hh:64 * hh + 64, kb * KB:(kb + 1) * KB],
                        rhs=qb[t][64 * hh:64 * hh + 64, qt * QT:(qt + 1) * QT],
                        start=True, stop=True,
                    )
                    pt = ppool.tile([KB, QT], BF16, tag="pt")
                    if causal and not diag:
                        nc.scalar.activation(out=pt, in_=sc, func=AF.Exp, scale=0.125)
                    else:
                        pat = (addpat[:, kb - 4 * qt, :] if causal else am)
                        ptf = ppool.tile([KB, QT], F32, tag="ptf")
                        nc.vector.scalar_tensor_tensor(
                            out=ptf, in0=sc, scalar=0.125, in1=pat,
                            op0=OP.mult, op1=OP.add)
                        nc.scalar.activation(out=pt, in_=ptf, func=AF.Exp)
                    nc.tensor.matmul(
                        av_ps[h][:, :],
                        lhsT=v_sb[:, kb, h, :],
                        rhs=pt[:, :],
                        start=(kb == 0), stop=(kb == nkb - 1),
                    )
            # normalize each head -> O^T
            for h in range(HPC):
                t, hh = h // 2, h % 2
                recip = opool.tile([65, QT], F32, tag="recip")
                nc.vector.reciprocal(recip, av_ps[h][:, :])
                bc_ps = ps_sc.tile([64, QT], F32, tag="sc")
                nc.tensor.matmul(bc_ps[:, :], lhsT=ones65[64:65, :],
                                 rhs=recip[64:65, :], start=True, stop=True)
                bc_sb = opool.tile([64, QT], F32, tag="bc")
                nc.scalar.activation(out=bc_sb, in_=bc_ps, func=AF.Copy)
                otmp = opool.tile([64, QT], BF16, tag="otmp")
                nc.vector.tensor_mul(otmp, av_ps[h][0:DK, :], bc_sb)
                nc.sync.dma_start(
                    out=oT[t][64 * hh:64 * hh + 64, qt * QT:(qt + 1) * QT],
                    in_=otmp)
            # --- output projection for this s-tile ---
            for jb in range(8):
                po = ps_sc.tile([128, QT], F32, tag="sc")
                for mb in range(2):
                    nc.tensor.matmul(
                        po[:, :],
                        lhsT=woT_sb[mb][:, jb * 128:(jb + 1) * 128],
                        rhs=oT[mb][:, qt * QT:(qt + 1) * QT],
                        start=(mb == 0), stop=(mb == 1),
                    )
                ob = opool.tile([128, QT], F32, tag="ob")
                nc.scalar.activation(out=ob, in_=po, func=AF.Copy)
                nc.sync.dma_start(
                    out=out_d[jb * 128:(jb + 1) * 128, qt * QT:(qt + 1) * QT],
                    in_=ob)

    with tile.TileContext(nc) as tc:
        emit(tc)
    nc.compile()
    return nc


def _host_prep(inputs):
    k, q, v = inputs["k"], inputs["q"], inputs["v"]
    mask, sin, cos = inputs["mask"], inputs["sin"], inputs["cos"]
    Wq, bq = inputs["Wq"], inputs["bq"]
    Wk, bk = inputs["Wk"], inputs["bk"]
    Wv, bv = inputs["Wv"], inputs["bv"]
    Wo = inputs["Wo"]

    causal = bool(np.array_equal(
        np.asarray(mask[0, 0]),
        np.tril(np.ones((S, S), np.asarray(mask).dtype))))

    sinT = np.ascontiguousarray(np.asarray(sin[0, 0]).T.astype(np.float32))
    cosT = np.ascontiguousarray(np.asarray(cos[0, 0]).T.astype(np.float32))
    sin4 = np.ascontiguousarray(np.tile(sinT, (4, 1)))
    cos4 = np.ascontiguousarray(np.tile(cosT, (4, 1)))

    if causal:
        ki = np.arange(KB)[:, None]
        qi = np.arange(QT)[None, :]
        pat = np.stack([
            np.where(128 * j + ki <= qi, 0.0, NEG).astype(np.float32)
            for j in range(4)])
        pat = np.ascontiguousarray(pat.transpose(1, 0, 2))
        amaskT = None
    else:
        pat = None
        amaskT = np.ascontiguousarray(
            np.where(np.asarray(mask[0, 0]).T == 0, NEG, 0.0).astype(np.float32))

    # E/O permutation of the 256 local head dims:
    # [all heads' even dims | all heads' odd dims]
    ev, od = np.arange(0, DK, 2), np.arange(1, DK, 2)
    perm = np.concatenate(
        [64 * h + ev for h in range(4)] + [64 * h + od for h in range(4)])

    xT = {}
    for name, x in (("q", q), ("k", k), ("v", v)):
        for b in range(B):
            xT[(name, b)] = np.ascontiguousarray(
                np.asarray(x[b]).T.astype(BF))

    in_maps = []
    for c in range(NCORES):
        b, hg = c // 4, c % 4
        rows = slice(hg * DH, (hg + 1) * DH)
        wqT = np.ascontiguousarray(np.asarray(Wq)[rows, :].T[:, perm])
        wkT = np.ascontiguousarray(np.asarray(Wk)[rows, :].T[:, perm])
        wvT = np.ascontiguousarray(np.asarray(Wv)[rows, :].T)
        woT = np.ascontiguousarray(np.asarray(Wo)[:, rows].T)
        m = dict(
            xqT=xT[("q", b)], xkT=xT[("k", b)], xvT=xT[("v", b)],
            wqT=wqT.astype(BF), wkT=wkT.astype(BF),
            wvT=wvT.astype(BF), woT=woT.astype(BF),
            bqA=np.ascontiguousarray(np.asarray(bq)[rows][perm]).astype(np.float32),
            bkA=np.ascontiguousarray(np.asarray(bk)[rows][perm]).astype(np.float32),
            bv=np.ascontiguousarray(np.asarray(bv)[rows]).astype(np.float32),
            sin4=sin4, cos4=cos4,
        )
        if causal:
            m["addpat"] = pat
        else:
            m["amaskT"] = amaskT
        in_maps.append(m)
    return causal, in_maps


def kernel(**inputs):
    from concourse.bass_utils import run_bass_kernel_spmd

    causal, in_maps = _host_prep(inputs)
    if causal not in _cache:
        _cache[causal] = _build_nc(causal)
    nc = _cache[causal]

    res = run_bass_kernel_spmd(nc, in_maps, core_ids=list(range(NCORES))).results

    bo = np.asarray(inputs["bo"]).astype(np.float32)
    out = np.empty((B, S, D), np.float32)
    for b in range(B):
        acc = res[4 * b]["outT"].astype(np.float32).copy()
        for c in range(4 * b + 1, 4 * b + 4):
            acc += res[c]["outT"]
        out[b] = acc.T + bo
    return out
